# revision 1
# baseline (speedup 1.0000x reference)
"""Trainium2 Bass kernel for KronLinear:
    out = x @ (sum_r kron(a_r, b_r)) + bias

Sharding: 2-way over tokens x 4-way over output columns across 8 cores.
fp8 e4m3 compute with DoubleRow perf mode (K=256 per matmul, 2 MACs per
PE cell per cycle), f32 PSUM accumulation.

Host: builds W = sum_r kron(a_r,b_r) (~2 GFLOP), scales by 256 so fp8
e4m3 stays in normal range, quantizes W and x to fp8, and pre-tiles
both in the DoubleRow-interleaved layout (contraction index
kappa = ktp*256 + 2*kp + ko).  Device: 32 m-tiles x 16 ktp x 2 matmuls
(N=512 out cols each, K=256), bias (pre-scaled x256) added on DVE, out
f32.  Host divides the gathered output by 256.
"""
import numpy as np

RANK = 64
A1 = A2 = B1 = B2 = 64
NTOK = 8192
NCORES = 8
TH = 2            # token shards
CQ = 4            # column shards
TOK_SH = NTOK // TH          # 4096 tokens per core
COLS_SH = (A2 * B2) // CQ    # 1024 out cols per core
JPC = A2 // CQ               # 16 j-values per core
MT = TOK_SH // 128           # 32 m-tiles
KTP = (A1 * B1) // 256       # 16 k-tile-pairs (K=256 each)
WSCALE = 256.0

_CACHE = {}


def _build_nc(debug=False):
    import sys
    if "/opt/trn_rl_repo" not in sys.path:
        sys.path.insert(0, "/opt/trn_rl_repo")
    import concourse.tile as tile
    from concourse import bacc, mybir

    f32 = mybir.dt.float32
    fp8 = mybir.dt.float8e4
    DR = mybir.MatmulPerfMode.DoubleRow

    nc = bacc.Bacc(None, target_bir_lowering=False, debug=debug,
                   num_devices=NCORES)

    # xt[mt, kp, ktp*256 + ko*128 + m] = x[mt*128+m, ktp*256 + 2*kp + ko]
    xt_d = nc.dram_tensor("xt", [MT, 128, KTP * 256], fp8, kind="ExternalInput")
    # wsl[ktp, kp, j*128 + ko*64 + l] = 256*W[ktp*256+2*kp+ko, j*64+l]
    w_d = nc.dram_tensor("wsl", [KTP, 128, 2 * COLS_SH], fp8,
                         kind="ExternalInput")
    bias_d = nc.dram_tensor("bias", [1, COLS_SH], f32, kind="ExternalInput")
    out_d = nc.dram_tensor("out", [TOK_SH, COLS_SH], f32, kind="ExternalOutput")

    with tile.TileContext(nc) as tc:
        with tc.tile_pool(name="const", bufs=1) as cpool, \
             tc.tile_pool(name="wres", bufs=1) as wpool, \
             tc.tile_pool(name="xin", bufs=3) as xpool, \
             tc.tile_pool(name="oout", bufs=2) as opool, \
             tc.tile_pool(name="mps", bufs=4, space="PSUM") as mps_pool:

            # Allocate W tiles first (keeps the same SBUF layout as v5/v7)
            # but issue the first two x-tile DMAs ahead of the W DMAs so
            # the PE's first matmuls aren't queued behind 4MB of W.
            w_sb = []
            for ktp in range(KTP):
                wt = wpool.tile([128, 2 * COLS_SH], fp8, tag=f"w{ktp}")
                w_sb.append(wt)
            bias_sb = cpool.tile([128, COLS_SH], f32)

            xts0 = xpool.tile([128, KTP * 256], fp8, tag="xts")
            nc.sync.dma_start(out=xts0[:], in_=xt_d[0, :, :])
            xts1 = xpool.tile([128, KTP * 256], fp8, tag="xts")
            nc.sync.dma_start(out=xts1[:], in_=xt_d[1, :, :])
            for ktp in range(KTP):
                nc.sync.dma_start(out=w_sb[ktp][:], in_=w_d[ktp, :, :])
            nc.sync.dma_start(
                out=bias_sb[:],
                in_=bias_d[:, :].broadcast_to([128, COLS_SH]))

            # ---- Main loop over token tiles.
            # mt 0 and 1 run their ktp loops interleaved: during startup
            # the W tiles trickle in from HBM slower than one m-tile
            # consumes them, so giving the PE two m-tiles of work per
            # arriving W tile removes the catch-up stall.
            def mm(ps, xts, ktp):
                lt = xts[:, ktp * 256:(ktp + 1) * 256] \
                    .rearrange("p (ko m) -> p ko m", ko=2)
                wv = w_sb[ktp][:, :] \
                    .rearrange("p (j ko l) -> p j ko l", ko=2, l=64)
                for h in range(2):
                    rhs = wv[:, 8 * h:8 * h + 8, :, :] \
                        .transpose([0, 2, 1, 3])
                    nc.tensor.matmul(
                        ps[:, 512 * h:512 * h + 512], lt, rhs,
                        start=(ktp == 0), stop=(ktp == KTP - 1),
                        perf_mode=DR)

            def tail(mt, ps, chunks=2):
                osb = opool.tile([128, COLS_SH], f32, tag="osb")
                cw = COLS_SH // chunks
                for h in range(chunks):
                    sl = slice(cw * h, cw * h + cw)
                    nc.vector.tensor_add(osb[:, sl], ps[:, sl],
                                         bias_sb[:, sl])
                    nc.sync.dma_start(
                        out=out_d[mt * 128:(mt + 1) * 128, sl],
                        in_=osb[:, sl])

            ps0 = mps_pool.tile([128, COLS_SH], f32, tag="ps")
            ps1 = mps_pool.tile([128, COLS_SH], f32, tag="ps")
            for ktp in range(KTP):
                mm(ps0, xts0, ktp)
                mm(ps1, xts1, ktp)
            tail(0, ps0)
            tail(1, ps1)

            for mt in range(2, MT):
                xts = xpool.tile([128, KTP * 256], fp8, tag="xts")
                nc.sync.dma_start(out=xts[:], in_=xt_d[mt, :, :])
                ps = mps_pool.tile([128, COLS_SH], f32, tag="ps")
                for ktp in range(KTP):
                    mm(ps, xts, ktp)
                # finer chunks on the last tile drain the DVE+DMA tail sooner
                tail(mt, ps, chunks=4 if mt == MT - 1 else 2)

    nc.compile()
    return nc


def _host_prep(x, a, b, bias):
    """Build per-core input maps. W built host-side, fp8 DR layouts."""
    import ml_dtypes
    f8 = ml_dtypes.float8_e4m3fn
    x = np.asarray(x, dtype=np.float32)
    a = np.asarray(a, dtype=np.float32)
    b = np.asarray(b, dtype=np.float32)
    bias = np.asarray(bias, dtype=np.float32)

    # W[(i,k),(j,l)] = sum_r a[r,i,j] b[r,k,l], scaled by WSCALE
    amat = a.transpose(1, 2, 0).reshape(A1 * A2, RANK)      # [(i,j), r]
    bmat = b.reshape(RANK, B1 * B2)                         # [r, (k,l)]
    wtmp = (amat @ bmat).reshape(A1, A2, B1, B2)            # [i, j, k, l]
    w = np.ascontiguousarray(wtmp.transpose(0, 2, 1, 3))    # [i, k, j, l]
    w = (w.reshape(A1 * B1, A2 * B2) * WSCALE).astype(f8)

    xt_by_th = []
    for th in range(TH):
        xh = x[th * TOK_SH:(th + 1) * TOK_SH]
        # xt[mt, kp, ktp, ko, m] = x[mt*128+m, ktp*256 + 2*kp + ko]
        x5 = xh.reshape(MT, 128, KTP, 128, 2)               # [mt, m, ktp, kp, ko]
        xt = np.ascontiguousarray(
            x5.transpose(0, 3, 2, 4, 1)).reshape(MT, 128, KTP * 256).astype(f8)
        xt_by_th.append(xt)
    w_by_cq = []
    bias_by_cq = []
    for cq in range(CQ):
        wsl = w[:, cq * COLS_SH:(cq + 1) * COLS_SH]         # [4096, 1024]
        # [ktp, kp, ko, j, l] -> [ktp, kp, j, ko, l]
        w5 = wsl.reshape(KTP, 128, 2, JPC, 64)
        w_by_cq.append(np.ascontiguousarray(
            w5.transpose(0, 1, 3, 2, 4)).reshape(KTP, 128, 2 * COLS_SH))
        bias_by_cq.append(np.ascontiguousarray(
            (bias[cq * COLS_SH:(cq + 1) * COLS_SH] * WSCALE)
            .reshape(1, COLS_SH)))

    in_maps = []
    for c in range(NCORES):
        th, cq = c // CQ, c % CQ
        in_maps.append({
            "xt": xt_by_th[th],
            "wsl": w_by_cq[cq],
            "bias": bias_by_cq[cq],
        })
    return in_maps


def kernel(x, a, b, bias):
    import sys
    if "/opt/trn_rl_repo" not in sys.path:
        sys.path.insert(0, "/opt/trn_rl_repo")
    from concourse.bass_utils import run_bass_kernel_spmd

    if "nc" not in _CACHE:
        _CACHE["nc"] = _build_nc(debug=False)
    nc = _CACHE["nc"]

    in_maps = _host_prep(x, a, b, bias)
    res = run_bass_kernel_spmd(nc, in_maps, core_ids=list(range(NCORES)))
    out = np.empty((NTOK, A2 * B2), dtype=np.float32)
    inv = np.float32(1.0 / WSCALE)
    for c in range(NCORES):
        th, cq = c // CQ, c % CQ
        np.multiply(res.results[c]["out"], inv,
                    out=out[th * TOK_SH:(th + 1) * TOK_SH,
                            cq * COLS_SH:(cq + 1) * COLS_SH])
    return out



# revision 2
# speedup vs baseline: 1.0065x; 1.0065x over previous
"""Trainium2 Bass kernel for KronLinear:
    out = x @ (sum_r kron(a_r, b_r)) + bias

Sharding: 2-way over tokens x 4-way over output columns across 8 cores.
fp8 e4m3 compute with DoubleRow perf mode (K=256 per matmul, 2 MACs per
PE cell per cycle), f32 PSUM accumulation.

Host: builds W = sum_r kron(a_r,b_r) (~2 GFLOP), scales by 256 so fp8
e4m3 stays in normal range, quantizes W and x to fp8, and pre-tiles
both in the DoubleRow-interleaved layout (contraction index
kappa = ktp*256 + 2*kp + ko).  Device: 32 m-tiles x 16 ktp x 2 matmuls
(N=512 out cols each, K=256), bias added on DVE, out f32.  Host divides
the gathered output by 256.

v2 vs baseline (252.6us):
 - startup: W-tile DMA issues split across the SP and Activation HWDGE
   engines (issue cost ~0.65us each serializes on one engine); x0 split
   so the first matmul's 32KB slice lands first; first matmul ~4us
   earlier.
 - PE warm-up: 5 dummy DoubleRow matmuls on a memset scratch tile run
   during the startup DMA window so the HAM clock gate (cold 1.2GHz ->
   warm 2.4GHz after ~3.4us of sustained PE activity) is released
   before the real matmul stream begins.
 - out DMA issues moved to the otherwise-idle Activation engine.
 - last m-tile runs its two 512-col PSUM halves as separate passes
   (h0 fully accumulated + drained while h1's matmuls run), and h1
   drains in 4x128-col chunks with issues alternating between the two
   HWDGE engines, cutting the end-of-kernel drain tail.
"""
import numpy as np

RANK = 64
A1 = A2 = B1 = B2 = 64
NTOK = 8192
NCORES = 8
TH = 2            # token shards
CQ = 4            # column shards
TOK_SH = NTOK // TH          # 4096 tokens per core
COLS_SH = (A2 * B2) // CQ    # 1024 out cols per core
JPC = A2 // CQ               # 16 j-values per core
MT = TOK_SH // 128           # 32 m-tiles
KTP = (A1 * B1) // 256       # 16 k-tile-pairs (K=256 each)
WSCALE = 256.0
NWARM = 5

_CACHE = {}


def _build_nc(debug=False):
    import sys
    if "/opt/trn_rl_repo" not in sys.path:
        sys.path.insert(0, "/opt/trn_rl_repo")
    import concourse.tile as tile
    from concourse import bacc, mybir

    f32 = mybir.dt.float32
    fp8 = mybir.dt.float8e4
    DR = mybir.MatmulPerfMode.DoubleRow

    nc = bacc.Bacc(None, target_bir_lowering=False, debug=debug,
                   num_devices=NCORES)

    # xt[mt, kp, ktp*256 + ko*128 + m] = x[mt*128+m, ktp*256 + 2*kp + ko]
    xt_d = nc.dram_tensor("xt", [MT, 128, KTP * 256], fp8, kind="ExternalInput")
    # wsl[ktp, kp, j*128 + ko*64 + l] = 256*W[ktp*256+2*kp+ko, j*64+l]
    w_d = nc.dram_tensor("wsl", [KTP, 128, 2 * COLS_SH], fp8,
                         kind="ExternalInput")
    bias_d = nc.dram_tensor("bias", [1, COLS_SH], f32, kind="ExternalInput")
    out_d = nc.dram_tensor("out", [TOK_SH, COLS_SH], f32, kind="ExternalOutput")

    with tile.TileContext(nc) as tc:
        with tc.tile_pool(name="const", bufs=1) as cpool, \
             tc.tile_pool(name="wres", bufs=1) as wpool, \
             tc.tile_pool(name="xin", bufs=3) as xpool, \
             tc.tile_pool(name="oout", bufs=2) as opool, \
             tc.tile_pool(name="mps", bufs=4, space="PSUM") as mps_pool:

            w_sb = []
            for ktp in range(KTP):
                wt = wpool.tile([128, 2 * COLS_SH], fp8, tag=f"w{ktp}")
                w_sb.append(wt)
            bias_sb = cpool.tile([128, COLS_SH], f32)
            scratch = cpool.tile([128, 1024], fp8, tag="warm")

            # ---- startup DMA issue schedule.  HWDGE issue costs ~0.65us
            # per instruction on the issuing engine, so split across the
            # two HWDGE engines (SP="sync", Activation="scalar") with the
            # first matmul's dependencies (x0's ktp0 slice + w0) first.
            xts0 = xpool.tile([128, KTP * 256], fp8, tag="xts")
            nc.sync.dma_start(out=xts0[:, 0:256], in_=xt_d[0, :, 0:256])
            nc.scalar.dma_start(out=w_sb[0][:], in_=w_d[0, :, :])
            nc.sync.dma_start(out=xts0[:, 256:], in_=xt_d[0, :, 256:])
            xts1 = xpool.tile([128, KTP * 256], fp8, tag="xts")
            nc.scalar.dma_start(out=xts1[:], in_=xt_d[1, :, :])
            for k in range(1, KTP):
                eng = nc.sync if (k % 2 == 1) else nc.scalar
                eng.dma_start(out=w_sb[k][:], in_=w_d[k, :, :])
            nc.sync.dma_start(
                out=bias_sb[:],
                in_=bias_d[:, :].broadcast_to([128, COLS_SH]))

            # ---- PE warm-up: memset scratch, then NWARM dummy DR matmuls
            # with no data deps beyond the memset.  These run during the
            # startup DMA window and release the HAM clock throttle.
            nc.gpsimd.memset(scratch[:], 0)
            ps_warm = mps_pool.tile([128, COLS_SH], f32, tag="ps")
            wl = scratch[:, 0:256].rearrange("p (ko m) -> p ko m", ko=2)
            wr = scratch[:, :].rearrange("p (ko n) -> p ko n", ko=2)
            for _ in range(NWARM):
                nc.tensor.matmul(ps_warm[:, 0:512], wl, wr,
                                 start=True, stop=True, perf_mode=DR)

            # ---- Main loop over token tiles.
            def mm(ps, xts, ktp, h, start=None, stop=None):
                lt = xts[:, ktp * 256:(ktp + 1) * 256] \
                    .rearrange("p (ko m) -> p ko m", ko=2)
                wv = w_sb[ktp][:, :] \
                    .rearrange("p (j ko l) -> p j ko l", ko=2, l=64)
                rhs = wv[:, 8 * h:8 * h + 8, :, :] \
                    .transpose([0, 2, 1, 3])
                nc.tensor.matmul(
                    ps[:, 512 * h:512 * h + 512], lt, rhs,
                    start=(ktp == 0) if start is None else start,
                    stop=(ktp == KTP - 1) if stop is None else stop,
                    perf_mode=DR)

            def drain(mt, ps, cols, chunks, engines):
                # add bias on DVE, then DMA out; issue on alternating
                # HWDGE engines.
                osb = opool.tile([128, COLS_SH], f32, tag="osb")
                lo, hi = cols
                cw = (hi - lo) // chunks
                for c in range(chunks):
                    sl = slice(lo + cw * c, lo + cw * c + cw)
                    nc.vector.tensor_add(osb[:, sl], ps[:, sl],
                                         bias_sb[:, sl])
                    engines[c % len(engines)].dma_start(
                        out=out_d[mt * 128:(mt + 1) * 128, sl],
                        in_=osb[:, sl])
                return osb

            # mt 0 and 1 interleaved (4 MMs per ktp) so the PE keeps pace
            # with the W tiles trickling in from HBM during startup.
            ps0 = mps_pool.tile([128, COLS_SH], f32, tag="ps")
            ps1 = mps_pool.tile([128, COLS_SH], f32, tag="ps")
            for ktp in range(KTP):
                mm(ps0, xts0, ktp, 0)
                mm(ps0, xts0, ktp, 1)
                mm(ps1, xts1, ktp, 0)
                mm(ps1, xts1, ktp, 1)
            drain(0, ps0, (0, COLS_SH), 2, [nc.scalar])
            drain(1, ps1, (0, COLS_SH), 2, [nc.scalar])

            for mt in range(2, MT - 1):
                xts = xpool.tile([128, KTP * 256], fp8, tag="xts")
                nc.sync.dma_start(out=xts[:], in_=xt_d[mt, :, :])
                ps = mps_pool.tile([128, COLS_SH], f32, tag="ps")
                for ktp in range(KTP):
                    mm(ps, xts, ktp, 0)
                    mm(ps, xts, ktp, 1)
                drain(mt, ps, (0, COLS_SH), 2, [nc.scalar])

            # Last m-tile: h0 accumulates and drains while h1's matmuls
            # still run; h1 drains in small chunks on both HWDGE engines
            # so almost nothing is left after the final matmul.
            mt = MT - 1
            xts = xpool.tile([128, KTP * 256], fp8, tag="xts")
            nc.sync.dma_start(out=xts[:], in_=xt_d[mt, :, :])
            ps = mps_pool.tile([128, COLS_SH], f32, tag="ps")
            for ktp in range(KTP):
                mm(ps, xts, ktp, 0)
            osb = opool.tile([128, COLS_SH], f32, tag="osb")
            for c in range(2):
                sl = slice(256 * c, 256 * c + 256)
                nc.vector.tensor_add(osb[:, sl], ps[:, sl], bias_sb[:, sl])
                nc.scalar.dma_start(
                    out=out_d[mt * 128:(mt + 1) * 128, sl], in_=osb[:, sl])
            for ktp in range(KTP):
                mm(ps, xts, ktp, 1)
            engs = [nc.scalar, nc.sync]
            for c in range(4):
                sl = slice(512 + 128 * c, 512 + 128 * c + 128)
                nc.vector.tensor_add(osb[:, sl], ps[:, sl], bias_sb[:, sl])
                engs[c % 2].dma_start(
                    out=out_d[mt * 128:(mt + 1) * 128, sl], in_=osb[:, sl])

    nc.compile()
    return nc


def _host_prep(x, a, b, bias):
    """Build per-core input maps. W built host-side, fp8 DR layouts."""
    import ml_dtypes
    f8 = ml_dtypes.float8_e4m3fn
    x = np.asarray(x, dtype=np.float32)
    a = np.asarray(a, dtype=np.float32)
    b = np.asarray(b, dtype=np.float32)
    bias = np.asarray(bias, dtype=np.float32)

    # W[(i,k),(j,l)] = sum_r a[r,i,j] b[r,k,l], scaled by WSCALE
    amat = a.transpose(1, 2, 0).reshape(A1 * A2, RANK)      # [(i,j), r]
    bmat = b.reshape(RANK, B1 * B2)                         # [r, (k,l)]
    wtmp = (amat @ bmat).reshape(A1, A2, B1, B2)            # [i, j, k, l]
    w = np.ascontiguousarray(wtmp.transpose(0, 2, 1, 3))    # [i, k, j, l]
    w = (w.reshape(A1 * B1, A2 * B2) * WSCALE).astype(f8)

    xt_by_th = []
    for th in range(TH):
        xh = x[th * TOK_SH:(th + 1) * TOK_SH]
        # xt[mt, kp, ktp, ko, m] = x[mt*128+m, ktp*256 + 2*kp + ko]
        x5 = xh.reshape(MT, 128, KTP, 128, 2)               # [mt, m, ktp, kp, ko]
        xt = np.ascontiguousarray(
            x5.transpose(0, 3, 2, 4, 1)).reshape(MT, 128, KTP * 256).astype(f8)
        xt_by_th.append(xt)
    w_by_cq = []
    bias_by_cq = []
    for cq in range(CQ):
        wsl = w[:, cq * COLS_SH:(cq + 1) * COLS_SH]         # [4096, 1024]
        # [ktp, kp, ko, j, l] -> [ktp, kp, j, ko, l]
        w5 = wsl.reshape(KTP, 128, 2, JPC, 64)
        w_by_cq.append(np.ascontiguousarray(
            w5.transpose(0, 1, 3, 2, 4)).reshape(KTP, 128, 2 * COLS_SH))
        bias_by_cq.append(np.ascontiguousarray(
            (bias[cq * COLS_SH:(cq + 1) * COLS_SH] * WSCALE)
            .reshape(1, COLS_SH)))

    in_maps = []
    for c in range(NCORES):
        th, cq = c // CQ, c % CQ
        in_maps.append({
            "xt": xt_by_th[th],
            "wsl": w_by_cq[cq],
            "bias": bias_by_cq[cq],
        })
    return in_maps


def kernel(x, a, b, bias):
    import sys
    if "/opt/trn_rl_repo" not in sys.path:
        sys.path.insert(0, "/opt/trn_rl_repo")
    from concourse.bass_utils import run_bass_kernel_spmd

    if "nc" not in _CACHE:
        _CACHE["nc"] = _build_nc(debug=False)
    nc = _CACHE["nc"]

    in_maps = _host_prep(x, a, b, bias)
    res = run_bass_kernel_spmd(nc, in_maps, core_ids=list(range(NCORES)))
    out = np.empty((NTOK, A2 * B2), dtype=np.float32)
    inv = np.float32(1.0 / WSCALE)
    for c in range(NCORES):
        th, cq = c // CQ, c % CQ
        np.multiply(res.results[c]["out"], inv,
                    out=out[th * TOK_SH:(th + 1) * TOK_SH,
                            cq * COLS_SH:(cq + 1) * COLS_SH])
    return out
